# revision 3
# baseline (speedup 1.0000x reference)
"""Trainium2 Bass kernel for nn_MultiHeadAttention_83056077570808.

GQA multi-head attention (32 q heads, 8 kv heads, d_head=128, T=2048,
D=4096) with RoPE, tanh soft-capping at 30, causal mask, fp32 reference.

Sharding: tensor-parallel over heads across 8 cores. Core c owns kv head c
and q heads 4c..4c+3: Wq/Wk/Wv column-sharded, Wo row-sharded; activations
replicated. Each core computes a partial output (its heads' contribution
through its Wo rows); the host sums the 8 partials.

Per-core pipeline (all layouts chosen so matmul contraction dim is the
partition dim):
  - host supplies query/key/value pre-transposed as X^T [D, T]
  - K/V proj: kT/vT [dk, T] = Wk/Wv-tile.T @ X^T   (f32r matmuls)
  - RoPE on kT and qT via a 128x128 rotation matmul + cos/sin elementwise
  - V transposed on-PE to V [T, dk], augmented with a ones column so the
    PV matmul computes the softmax denominator for free
  - Q proj per 512-wide t-chunk, then per q head:
      S^T[Tt, t] = kT_rope-tile.T @ qT_rope      (f32r, N=512)
      P^T = exp(30*tanh(S^T * scale/30)) via two ACT passes (scales fused),
      bf16 output; causal mask applied only on diagonal tile groups
      attn[t, 0:129] = P^T-slice.T @ [V | ones]  (bf16, N=129, accumulated
      over T tiles; col 128 = denominator)
      normalize by 1/denom (per-partition scalar), PE-transpose to
      attnT [dk, t], spill to a DRAM scratch tile
  - O proj: out[t, :] += attnT-tile.T @ Wo-tile  (f32r, N=512)

No max-subtraction is needed in the softmax: soft-capping bounds logits to
[-30, 30], so exp() is safely within fp32 range.
"""

import os
import sys

for _p in ("/opt/trn_rl_repo", os.path.expanduser("~/.axon_site/_ro/trn_rl_repo")):
    if os.path.isdir(_p) and _p not in sys.path:
        sys.path.insert(0, _p)

import numpy as np
import ml_dtypes

import concourse.bass as bass
import concourse.tile as tile
from concourse import bacc, mybir
from concourse.bass_utils import run_bass_kernel_spmd

F32 = mybir.dt.float32
F32R = mybir.dt.float32r
BF16 = mybir.dt.bfloat16

D_MODEL = 4096
KEY_SIZE = 128
NUM_Q_HEADS = 32
NUM_KV_HEADS = 8
N_CORES = 8
NH = NUM_Q_HEADS // NUM_KV_HEADS  # q heads per core = 4
ATTN_MULT = 0.08838834764831845
CAP = 30.0

Tanh = mybir.ActivationFunctionType.Tanh
Exp = mybir.ActivationFunctionType.Exp


def build_nc(T: int, causal: bool):
    """Emit the Bass program for one core (SPMD: all cores run this)."""
    D = D_MODEL
    TC = 512                 # t-chunk width for attention
    NTC = T // TC            # t-chunks
    NTT = T // 128           # T tiles (key side)
    NDT = D // 128           # contraction tiles over d_model = 32
    JW = NH * KEY_SIZE       # per-core q/o width = 512

    nc = bacc.Bacc(None, target_bir_lowering=False)

    xq = nc.dram_tensor("xq", [D, T], F32R, kind="ExternalInput")
    xk = nc.dram_tensor("xk", [D, T], F32R, kind="ExternalInput")
    xv = nc.dram_tensor("xv", [D, T], F32R, kind="ExternalInput")
    wq = nc.dram_tensor("wq", [D, JW], F32R, kind="ExternalInput")
    wk = nc.dram_tensor("wk", [D, KEY_SIZE], F32R, kind="ExternalInput")
    wv = nc.dram_tensor("wv", [D, KEY_SIZE], F32R, kind="ExternalInput")
    wo = nc.dram_tensor("wo", [JW, D], F32R, kind="ExternalInput")
    cosd = nc.dram_tensor("cosT", [128, T], F32, kind="ExternalInput")
    sind = nc.dram_tensor("sinT", [128, T], F32, kind="ExternalInput")
    rotd = nc.dram_tensor("rot", [128, 128], F32R, kind="ExternalInput")
    identd = nc.dram_tensor("ident", [128, 128], F32R, kind="ExternalInput")
    maskdd = nc.dram_tensor("maskd", [128, 4 * TC], BF16, kind="ExternalInput")
    vbgd = nc.dram_tensor("vbg", [128, NTT, 4], BF16, kind="ExternalInput")
    outd = nc.dram_tensor("out", [T, D], F32, kind="ExternalOutput")

    with tile.TileContext(nc) as tc:
        with (
            tc.tile_pool(name="const", bufs=1) as constp,
            tc.tile_pool(name="persist", bufs=1) as persist,
            tc.tile_pool(name="dram", bufs=1, space="DRAM") as dramp,
            tc.tile_pool(name="qkps", bufs=1, space="PSUM") as qkps,
            tc.tile_pool(name="pvps", bufs=1, space="PSUM") as pvps,
            tc.tile_pool(name="mmps", bufs=2, space="PSUM") as mmps,
        ):
            rot_sb = constp.tile([128, 128], F32R)
            ident_sb = constp.tile([128, 128], F32R)
            cos_sb = constp.tile([128, T], F32)
            sin_sb = constp.tile([128, T], F32)
            mask_sb = constp.tile([128, 4, TC], BF16)
            nc.sync.dma_start(out=rot_sb, in_=rotd[:])
            nc.sync.dma_start(out=ident_sb, in_=identd[:])
            nc.sync.dma_start(out=cos_sb, in_=cosd[:])
            nc.sync.dma_start(out=sin_sb, in_=sind[:])
            nc.sync.dma_start(
                out=mask_sb, in_=maskdd.rearrange("k (b t) -> k b t", b=4)
            )

            kT_rope = persist.tile([128, T], F32R)
            vaug = persist.tile([128, NTT, 132], BF16)
            nc.sync.dma_start(out=vaug[:, :, 128:132], in_=vbgd[:])

            attnT_d = dramp.tile([NH, 128, T], F32R)

            def rope(dst, src, t0, tw):
                """dst[128, tw] (f32r) = RoPE(src[128, tw]) at positions t0.."""
                rp = mmps.tile([128, 512], F32, tag="mm", name="rope_ps")
                nc.tensor.matmul(
                    rp[:, :tw], rot_sb, src, start=True, stop=True
                )
                t1 = ropetmp.tile([128, 512], F32, tag="rt", name="rope_t1")
                nc.vector.tensor_mul(
                    t1[:, :tw], src.bitcast(F32), cos_sb[:, t0 : t0 + tw]
                )
                t2 = ropetmp.tile([128, 512], F32, tag="rt", name="rope_t2")
                nc.vector.tensor_mul(t2[:, :tw], rp[:, :tw], sin_sb[:, t0 : t0 + tw])
                nc.vector.tensor_add(dst, t1[:, :tw], t2[:, :tw])

            # ---------------- phase 1: K/V projections, RoPE-k, V transpose
            with (
                tc.tile_pool(name="ph1", bufs=1) as ph1,
                tc.tile_pool(name="ph1slab", bufs=2) as ph1slab,
                tc.tile_pool(name="ropetmp", bufs=2) as ropetmp,
            ):
                wk_sb = ph1.tile([128, NDT, 128], F32R)
                wv_sb = ph1.tile([128, NDT, 128], F32R)
                nc.sync.dma_start(
                    out=wk_sb, in_=wk.rearrange("(n k) j -> k n j", k=128)
                )
                nc.sync.dma_start(
                    out=wv_sb, in_=wv.rearrange("(n k) j -> k n j", k=128)
                )
                kproj = ph1.tile([128, T], F32R)
                vproj = ph1.tile([128, T], F32R)

                for w_sb, xsrc, dest, sname in (
                    (wk_sb, xk, kproj, "kproj"),
                    (wv_sb, xv, vproj, "vproj"),
                ):
                    with nc.named_scope(sname):
                        for tch in range(T // 256):
                            ps = mmps.tile([128, 512], F32, tag="mm", name="kv_ps")
                            for dh in range(2):
                                slab = ph1slab.tile(
                                    [128, 16, 256], F32R, tag="slab", name="kvslab"
                                )
                                nc.sync.dma_start(
                                    out=slab,
                                    in_=xsrc[
                                        dh * 2048 : (dh + 1) * 2048,
                                        tch * 256 : (tch + 1) * 256,
                                    ].rearrange("(n k) t -> k n t", k=128),
                                )
                                for i in range(16):
                                    nc.tensor.matmul(
                                        ps[:, :256],
                                        w_sb[:, dh * 16 + i, :],
                                        slab[:, i, :],
                                        start=(dh == 0 and i == 0),
                                        stop=(dh == 1 and i == 15),
                                    )
                            nc.scalar.copy(
                                out=dest[:, tch * 256 : (tch + 1) * 256],
                                in_=ps[:, :256],
                            )

                with nc.named_scope("ropek"):
                    for ch in range(NTC):
                        rope(
                            kT_rope[:, ch * TC : (ch + 1) * TC],
                            kproj[:, ch * TC : (ch + 1) * TC],
                            ch * TC,
                            TC,
                        )
                with nc.named_scope("vtrans"):
                    for b in range(NTT):
                        tp = mmps.tile([128, 512], F32R, tag="mm", name="vtr_ps")
                        nc.tensor.transpose(
                            tp[:, :128],
                            vproj[:, b * 128 : (b + 1) * 128],
                            ident_sb,
                        )
                        nc.vector.tensor_copy(
                            vaug[:, b, 0:128], tp[:, :128].bitcast(F32)
                        )

            # ---------------- pass A: Q proj + RoPE-q + attention per t-chunk
            with (
                tc.tile_pool(name="pa", bufs=1) as pa,
                tc.tile_pool(name="paslab", bufs=2) as paslab,
                tc.tile_pool(name="ropetmp", bufs=2) as ropetmp,
            ):
                wq_sb = pa.tile([128, NDT, JW], F32R)
                nc.sync.dma_start(
                    out=wq_sb, in_=wq.rearrange("(n k) j -> k n j", k=128)
                )
                for tcx in range(NTC):
                    t0 = tcx * TC
                    with nc.named_scope("qproj"):
                        qraw = pa.tile([128, NH, TC], F32R, tag="qraw", bufs=2)
                        for th in range(TC // 256):
                            slabs = []
                            for dh in range(2):
                                slab = paslab.tile(
                                    [128, 16, 256], F32R, tag="qslab", name="qslab"
                                )
                                nc.sync.dma_start(
                                    out=slab,
                                    in_=xq[
                                        dh * 2048 : (dh + 1) * 2048,
                                        t0 + th * 256 : t0 + (th + 1) * 256,
                                    ].rearrange("(n k) t -> k n t", k=128),
                                )
                                slabs.append(slab)
                            for jh in range(NH):
                                ps = mmps.tile([128, 512], F32, tag="mm", name="q_ps")
                                for dh in range(2):
                                    for i in range(16):
                                        nc.tensor.matmul(
                                            ps[:, :256],
                                            wq_sb[
                                                :,
                                                dh * 16 + i,
                                                jh * 128 : (jh + 1) * 128,
                                            ],
                                            slabs[dh][:, i, :],
                                            start=(dh == 0 and i == 0),
                                            stop=(dh == 1 and i == 15),
                                        )
                                nc.scalar.copy(
                                    out=qraw[:, jh, th * 256 : (th + 1) * 256],
                                    in_=ps[:, :256],
                                )
                    with nc.named_scope("ropeq"):
                        qrope = pa.tile([128, NH, TC], F32R, tag="qrope", bufs=2)
                        for jh in range(NH):
                            rope(qrope[:, jh, :], qraw[:, jh, :], t0, TC)

                    ngroups = (tcx + 1) if causal else (NTT // 4)
                    for h in range(NH):
                        with nc.named_scope("attn"):
                            pv = pvps.tile([128, 4, 256], F32, tag="pv", name="pv_ps")
                            for g in range(ngroups):
                                qk = qkps.tile(
                                    [128, 4, 512], F32, tag="qk", name="qk_ps"
                                )
                                for b in range(4):
                                    Tt = 4 * g + b
                                    nc.tensor.matmul(
                                        qk[:, b, :],
                                        kT_rope[:, Tt * 128 : (Tt + 1) * 128],
                                        qrope[:, h, :],
                                        start=True,
                                        stop=True,
                                    )
                                st = pa.tile(
                                    [128, 4, TC], F32, tag="stmp", bufs=1, name="stmp"
                                )
                                nc.scalar.activation(
                                    out=st, in_=qk, func=Tanh, scale=ATTN_MULT / CAP
                                )
                                pt = pa.tile(
                                    [128, 4, TC], BF16, tag="pt", bufs=2, name="ptile"
                                )
                                nc.scalar.activation(
                                    out=pt, in_=st, func=Exp, scale=CAP
                                )
                                if causal and g == tcx:
                                    nc.vector.tensor_mul(pt, pt, mask_sb)
                                for s in range(4):
                                    for b in range(4):
                                        Tt = 4 * g + b
                                        # start=True clears has_written for the
                                        # WHOLE psum bank; two s-chains share a
                                        # bank, so only the bank's first chain
                                        # may clear. The sibling chain's first
                                        # write lands on cleared bits and
                                        # overwrites (no clear needed).
                                        nc.tensor.matmul(
                                            pv[:, s, 0:129],
                                            pt[:, b, s * 128 : (s + 1) * 128],
                                            vaug[:, Tt, 0:129],
                                            start=(g == 0 and b == 0 and s % 2 == 0),
                                            stop=(g == ngroups - 1 and b == 3),
                                            skip_group_check=True,
                                        )
                        with nc.named_scope("attn_fin"):
                            aT_sb = pa.tile(
                                [128, TC], F32R, tag="atsb", bufs=2, name="atsb"
                            )
                            for s in range(4):
                                rc = pa.tile(
                                    [128, 1], F32, tag="rc", bufs=4, name="rc"
                                )
                                nc.vector.reciprocal(rc, pv[:, s, 128:129])
                                an = pa.tile(
                                    [128, 128], F32R, tag="an", bufs=2, name="an"
                                )
                                nc.vector.tensor_scalar_mul(an, pv[:, s, 0:128], rc)
                                tp = mmps.tile(
                                    [128, 512], F32R, tag="mm", name="atr_ps"
                                )
                                nc.tensor.transpose(tp[:, :128], an, ident_sb)
                                nc.vector.tensor_copy(
                                    aT_sb[:, s * 128 : (s + 1) * 128], tp[:, :128]
                                )
                            nc.sync.dma_start(
                                out=attnT_d[h, :, t0 : t0 + TC], in_=aT_sb
                            )

            # ---------------- pass B: O projection
            with tc.tile_pool(name="pb", bufs=1) as pb:
                wo_sb = pb.tile([128, NH, D], F32R)
                nc.sync.dma_start(
                    out=wo_sb, in_=wo.rearrange("(n k) d -> k n d", k=128)
                )
                with nc.named_scope("oproj"):
                    for tt in range(T // 128):
                        aT = pb.tile([128, NH, 128], F32R, tag="aT", bufs=2, name="aT")
                        nc.sync.dma_start(
                            out=aT,
                            in_=attnT_d[:, :, tt * 128 : (tt + 1) * 128].rearrange(
                                "h k t -> k h t"
                            ),
                        )
                        for nch in range(D // 512):
                            ps = mmps.tile([128, 512], F32, tag="mm", name="o_ps")
                            for jh in range(NH):
                                nc.tensor.matmul(
                                    ps,
                                    aT[:, jh, :],
                                    wo_sb[:, jh, nch * 512 : (nch + 1) * 512],
                                    start=(jh == 0),
                                    stop=(jh == NH - 1),
                                )
                            osb = pb.tile(
                                [128, 512], F32, tag="osb", bufs=3, name="osb"
                            )
                            nc.vector.tensor_copy(osb, ps)
                            nc.sync.dma_start(
                                out=outd[
                                    tt * 128 : (tt + 1) * 128,
                                    nch * 512 : (nch + 1) * 512,
                                ],
                                in_=osb,
                            )

    nc.compile()
    return nc


def _host_constants(T: int):
    d = KEY_SIZE
    inv_freq = (1.0 / (10000.0 ** (np.arange(0, d, 2, dtype=np.float64) / d))).astype(
        np.float64
    )  # [64]
    pos = np.arange(T, dtype=np.float64)
    phase_half = pos[None, :] * inv_freq[:, None]  # [64, T]
    phase = np.concatenate([phase_half, phase_half], axis=0)  # [128, T] (tiled)
    cosT = np.cos(phase).astype(np.float32)
    sinT = np.sin(phase).astype(np.float32)

    R = np.zeros((128, 128), dtype=np.float32)
    R[:64, 64:] = -np.eye(64, dtype=np.float32)
    R[64:, :64] = np.eye(64, dtype=np.float32)
    rot = np.ascontiguousarray(R.T)

    ident = np.eye(128, dtype=np.float32)

    TC = 512
    tl = np.arange(TC)
    Tl = np.arange(128)
    maskd = np.zeros((128, 4, TC), dtype=np.float32)
    for b in range(4):
        maskd[:, b, :] = (128 * b + Tl[:, None]) <= tl[None, :]
    maskd = maskd.reshape(128, 4 * TC).astype(ml_dtypes.bfloat16)

    NTT = T // 128
    vbg = np.zeros((128, NTT, 4), dtype=ml_dtypes.bfloat16)
    vbg[:, :, 0] = 1.0
    return cosT, sinT, rot, ident, maskd, vbg


_NC_CACHE: dict = {}
LAST_RESULT = None
_LAST_IN_MAPS = None


def kernel(query, key, value, mask, Wq, Wk, Wv, Wo):
    global LAST_RESULT
    query = np.asarray(query)
    key = np.asarray(key)
    value = np.asarray(value)
    mask = np.asarray(mask)
    Wq = np.asarray(Wq, dtype=np.float32)
    Wk = np.asarray(Wk, dtype=np.float32)
    Wv = np.asarray(Wv, dtype=np.float32)
    Wo = np.asarray(Wo, dtype=np.float32)

    b, T, D = query.shape
    assert b == 1 and D == D_MODEL, (b, D)

    m2 = np.asarray(mask).reshape(T, T).astype(bool)
    if np.array_equal(m2, np.tril(np.ones((T, T), dtype=bool))):
        causal = True
    elif m2.all():
        causal = False
    else:
        raise ValueError("unsupported mask pattern (expected causal or full)")

    kkey = (T, causal)
    if kkey not in _NC_CACHE:
        _NC_CACHE[kkey] = build_nc(T, causal)
    nc = _NC_CACHE[kkey]

    xq = np.ascontiguousarray(query[0].T.astype(np.float32))  # [D, T]
    xk = np.ascontiguousarray(key[0].T.astype(np.float32))
    xv = np.ascontiguousarray(value[0].T.astype(np.float32))
    cosT, sinT, rot, ident, maskd, vbg = _host_constants(T)

    JW = NH * KEY_SIZE
    in_maps = []
    for c in range(N_CORES):
        in_maps.append(
            {
                "xq": xq,
                "xk": xk,
                "xv": xv,
                "wq": np.ascontiguousarray(Wq[:, c * JW : (c + 1) * JW]),
                "wk": np.ascontiguousarray(Wk[:, c * KEY_SIZE : (c + 1) * KEY_SIZE]),
                "wv": np.ascontiguousarray(Wv[:, c * KEY_SIZE : (c + 1) * KEY_SIZE]),
                "wo": np.ascontiguousarray(Wo[c * JW : (c + 1) * JW, :]),
                "cosT": cosT,
                "sinT": sinT,
                "rot": rot,
                "ident": ident,
                "maskd": maskd,
                "vbg": vbg,
            }
        )

    global _LAST_IN_MAPS
    _LAST_IN_MAPS = in_maps
    trace = os.environ.get("MHA_TRACE") == "1"
    res = run_bass_kernel_spmd(nc, in_maps, list(range(N_CORES)), trace=trace)
    LAST_RESULT = res

    out = np.zeros((T, D), dtype=np.float64)
    for c in range(N_CORES):
        out += res.results[c]["out"].astype(np.float64)
    return out.astype(np.float32).reshape(1, T, D)
